# revision 14
# baseline (speedup 1.0000x reference)
"""Trainium2 Bass kernel for nn_DeepLabHeadV3Plus (8-core SPMD, batch-sharded).

Strategy
--------
Data-parallel over batch: core i owns sample i (B=8, 8 cores). Training-mode
BatchNorm statistics are all-reduced across cores. The PSO channel-selection
loop (inherently sequential, 900 tiny steps) runs on the host between two
device stages, replicated bit-for-bit with jax-on-CPU:

  Stage A (device): low-projection conv, ASPP branch convs (1x1, three dilated
      3x3, pooled 1x1, w_max 1x1), BN stats all-reduce, BN+ReLU, xm,
      per-channel partial sums for PSO, and the r0..r4 part of the 1x1
      projection (acc5).
  Host: global ch_sums, PSO -> best[9], x_sel gather from xm.
  Stage B (device): r5 conv + proj, aspp BN (AR), bilinear x2 upsample,
      concat with low, 3x3 classifier conv + BN (AR), final 1x1 conv + bias.

Matmuls run as float32r (fp32 with hardware RNE rounding to 11 mantissa bits
at the PE input, full-rate for N>=256). The w_max conv stays in exact fp32 so
the discrete PSO selection is numerically stable.
"""
import os
import sys

for _p in ("/opt/trn_rl_repo", "/root/.axon_site/_ro/trn_rl_repo"):
    if os.path.isdir(_p) and _p not in sys.path:
        sys.path.insert(0, _p)
        break

import numpy as np

import concourse.bass as bass  # noqa: F401  (engine types via nc)
import concourse.mybir as mybir
import concourse.tile as tile
from concourse import bacc
from concourse.bass_utils import run_bass_kernel_spmd

F32 = mybir.dt.float32
F32R = mybir.dt.float32r
AF = mybir.ActivationFunctionType
ALU = mybir.AluOpType
AX = mybir.AxisListType

P = 128
NCORES = 8
HW = 4096            # 64*64
W64 = 64
KT = 16              # 2048 / 128 cin tiles
NB = 8               # pixel blocks of 512 over 64x64
DILS = (12, 24, 36)
EPS = 1e-5
N_HW = 8 * HW        # BN count for 64x64 maps (batch*spatial)
N_LOW = 8 * 16384    # BN count for 128x128 maps
N_POOL = 8           # BN count for pooled branch

RG = [list(range(NCORES))]

_trace = bool(int(os.environ.get("TRN_TRACE", "0")))
LAST_EXEC_NS = {}


# --------------------------------------------------------------------------
# builder helpers
# --------------------------------------------------------------------------

def _bn_cols(nc, pool, statsg, s_cols, i_inst, cS, cQ, n, rows=P):
    """From global-sum cols statsg[:,cS] (sum) and statsg[:,cQ] (sum of sq),
    write scale into s_cols[:, 2*i] and bias into s_cols[:, 2*i+1]:
      s = 1/sqrt(var+eps), b = -mean*s."""
    m = pool.tile([P, 1], F32, name=f"bn_m_{i_inst}")
    v = pool.tile([P, 1], F32, name=f"bn_v_{i_inst}")
    t = pool.tile([P, 1], F32, name=f"bn_t_{i_inst}")
    r = slice(0, rows)
    sc = s_cols[r, 2 * i_inst:2 * i_inst + 1]
    bc = s_cols[r, 2 * i_inst + 1:2 * i_inst + 2]
    nc.vector.tensor_scalar_mul(m[r, :], statsg[r, cS:cS + 1], 1.0 / n)
    nc.vector.tensor_scalar_mul(v[r, :], statsg[r, cQ:cQ + 1], 1.0 / n)
    nc.vector.tensor_mul(t[r, :], m[r, :], m[r, :])
    nc.vector.tensor_sub(v[r, :], v[r, :], t[r, :])
    nc.vector.tensor_scalar_add(v[r, :], v[r, :], EPS)
    nc.vector.reciprocal(t[r, :], v[r, :])
    nc.scalar.activation(sc, t[r, :], AF.Sqrt)
    nc.vector.tensor_mul(bc, m[r, :], sc)
    nc.vector.tensor_scalar_mul(bc, bc, -1.0)


def build_stage_a():
    nc = bacc.Bacc("TRN2", target_bir_lowering=False, debug=False)

    x_d = nc.dram_tensor("x", [2048, HW], F32, kind="ExternalInput")
    wdil_d = nc.dram_tensor("wdil", [2048, 3 * 9 * 256], F32, kind="ExternalInput")
    wa0_d = nc.dram_tensor("wa0", [2048, 256], F32, kind="ExternalInput")
    wmax_d = nc.dram_tensor("wmax", [2048, 256], F32, kind="ExternalInput")
    wpool_d = nc.dram_tensor("wpool", [2048, 256], F32, kind="ExternalInput")
    xlow_d = nc.dram_tensor("xlow", [256, 16384], F32, kind="ExternalInput")
    wlow_d = nc.dram_tensor("wlow", [256, 48], F32, kind="ExternalInput")
    wproj_d = nc.dram_tensor("wproj", [1536, 256], F32, kind="ExternalInput")

    xm_o = nc.dram_tensor("xm_o", [256, HW], F32, kind="ExternalOutput")
    chs_o = nc.dram_tensor("chs_o", [256, 1], F32, kind="ExternalOutput")
    acc5_o = nc.dram_tensor("acc5_o", [256, HW], F32, kind="ExternalOutput")
    low_o = nc.dram_tensor("low_o", [48, 16384], F32, kind="ExternalOutput")

    with tile.TileContext(nc) as tc:
        glob_cm = tc.tile_pool(name="glob", bufs=1)
        dram_cm = tc.tile_pool(name="dramp", bufs=1, space="DRAM")
        glob = glob_cm.__enter__()
        dram = dram_cm.__enter__()

        ydil = dram.tile([3, 256, HW], F32, name="ydil")
        y0_dr = dram.tile([256, HW], F32, name="y0_dr")
        ymax_dr = dram.tile([256, HW], F32, name="ymax_dr")
        ylow_dr = dram.tile([48, 16384], F32, name="ylow_dr")
        stats_in = dram.tile([P, 26], F32, name="stats_in")
        stats_out = dram.tile([P, 26], F32, name="stats_out")

        # stat columns (local sums), packed for one all-reduce
        # 0..11  : dilated   [br*4 + (S0,S1,Q0,Q1)]
        # 12..15 : y0        (S0,S1,Q0,Q1)
        # 16..19 : ymax
        # 20,21  : low       (S,Q) rows 0:48
        # 22..25 : ypool     (v0,v1,q0,q1)
        stats = glob.tile([P, 26], F32, name="stats")
        statsg = glob.tile([P, 26], F32, name="statsg")
        pooled_cols = glob.tile([P, KT * NB], F32, name="pooled_cols")

        # ---------------- phase 0c: low-level projection conv ----------------
        with (
            tc.tile_pool(name="p0c", bufs=1) as pc,
            tc.tile_pool(name="p0c_ps", bufs=1, space="PSUM") as pcps,
        ):
            wlt = pc.tile([P, 2, 48], F32R, name="wlt")
            nc.sync.dma_start(
                wlt[:], wlow_d[:, :].rearrange("(t p) c -> p t c", p=P).bitcast(F32R))
            lowS = glob.tile([P, 32], F32, name="lowS")
            lowQ = glob.tile([P, 32], F32, name="lowQ")
            for pb in range(32):
                pslow = pcps.tile([P, 512], F32, name="pslow")
                for k in range(2):
                    xlt = pc.tile([P, 512], F32R, name="xlt", bufs=3)
                    nc.sync.dma_start(
                        xlt[:],
                        xlow_d[k * P:(k + 1) * P, pb * 512:(pb + 1) * 512].bitcast(F32R))
                    nc.tensor.matmul(pslow[0:48, :], wlt[:, k, :], xlt[:],
                                     start=(k == 0), stop=(k == 1))
                st = pc.tile([P, 512], F32, name="st_low", bufs=2)
                nc.scalar.activation(st[0:48, :], pslow[0:48, :], AF.Identity,
                                     accum_out=lowS[0:48, pb:pb + 1])
                sq = pc.tile([P, 512], F32, name="sq_low", bufs=2)
                nc.scalar.activation(sq[0:48, :], pslow[0:48, :], AF.Square,
                                     accum_out=lowQ[0:48, pb:pb + 1])
                nc.sync.dma_start(ylow_dr[:, pb * 512:(pb + 1) * 512], st[0:48, :])
            nc.vector.reduce_sum(stats[0:48, 20:21], lowS[0:48, :], axis=AX.X)
            nc.vector.reduce_sum(stats[0:48, 21:22], lowQ[0:48, :], axis=AX.X)

        # ---------------- phase 0a: dilated 3x3 convs (k-outer) ----------------
        with (
            tc.tile_pool(name="p0a_acc", bufs=1) as pacc,
            tc.tile_pool(name="p0a_s", bufs=1) as ps_,
            tc.tile_pool(name="p0a_ps", bufs=1, space="PSUM") as pps,
        ):
            accs = {}
            for br in range(3):
                for ct in range(2):
                    accs[(br, ct)] = pacc.tile([P, HW], F32, name=f"acc{br}{ct}")
            for k in range(KT):
                wk = ps_.tile([P, 27 * 256], F32R, name="wk", bufs=2)
                nc.sync.dma_start(
                    wk[:], wdil_d[k * P:(k + 1) * P, :].bitcast(F32R))
                wk4 = wk.rearrange("p (b t c) -> p b t c", b=3, t=9)
                xk = ps_.tile([P, HW], F32R, name="xk", bufs=2)
                nc.sync.dma_start(xk[:], x_d[k * P:(k + 1) * P, :].bitcast(F32R))
                xk3 = xk.rearrange("p (h w) -> p h w", h=W64)
                for rb in range(8):
                    y0r = rb * 8
                    for br, d in enumerate(DILS):
                        for ct in range(2):
                            ps = pps.tile([P, 512], F32, name=f"ps{br}{ct}")
                            ps3 = ps.rearrange("p (h w) -> p h w", h=8)
                            taps = []
                            for iy, dy in enumerate((-d, 0, d)):
                                ylo = max(y0r, -dy)
                                yhi = min(y0r + 8, W64 - dy)
                                if yhi <= ylo:
                                    continue
                                for ix, dx in enumerate((-d, 0, d)):
                                    xlo = max(0, -dx)
                                    xhi = min(W64, W64 - dx)
                                    taps.append((iy * 3 + ix, dy, dx, ylo, yhi, xlo, xhi))
                            nt = len(taps)
                            for i, (t, dy, dx, ylo, yhi, xlo, xhi) in enumerate(taps):
                                nc.tensor.matmul(
                                    ps3[:, ylo - y0r:yhi - y0r, xlo:xhi],
                                    wk4[:, br, t, ct * P:(ct + 1) * P],
                                    xk3[:, ylo + dy:yhi + dy, xlo + dx:xhi + dx],
                                    start=(i == 0), stop=(i == nt - 1))
                            dst = accs[(br, ct)][:, y0r * W64:(y0r + 8) * W64]
                            if k == 0:
                                nc.scalar.activation(dst, ps[:], AF.Identity)
                            else:
                                nc.vector.tensor_add(dst, dst, ps[:])
            # stats + spill y to DRAM
            for br in range(3):
                for ct in range(2):
                    a = accs[(br, ct)]
                    st = ps_.tile([P, HW], F32, name="sq_dil", bufs=1)
                    nc.scalar.activation(st[:], a[:], AF.Square,
                                         accum_out=stats[:, br * 4 + 2 + ct:br * 4 + 3 + ct])
                    nc.vector.reduce_sum(stats[:, br * 4 + ct:br * 4 + 1 + ct],
                                         a[:], axis=AX.X)
                    nc.sync.dma_start(ydil[br, ct * P:(ct + 1) * P, :], a[:])

        # ---------------- phase 0b: 1x1 convs (w_a0 f32r, w_max fp32) ----------------
        with (
            tc.tile_pool(name="p0b_w", bufs=1) as pw,
            tc.tile_pool(name="p0b_s", bufs=1) as pbs,
            tc.tile_pool(name="p0b_ps", bufs=1, space="PSUM") as pbps,
        ):
            w0t = pw.tile([P, KT, 256], F32R, name="w0t")
            nc.sync.dma_start(
                w0t[:], wa0_d[:, :].rearrange("(t p) c -> p t c", p=P).bitcast(F32R))
            wmt = pw.tile([P, KT, 256], F32, name="wmt")
            nc.sync.dma_start(
                wmt[:], wmax_d[:, :].rearrange("(t p) c -> p t c", p=P))
            wpt = pw.tile([P, KT, 256], F32, name="wpt")
            nc.sync.dma_start(
                wpt[:], wpool_d[:, :].rearrange("(t p) c -> p t c", p=P))

            y0S = glob.tile([P, 2 * NB], F32, name="y0S")
            y0Q = glob.tile([P, 2 * NB], F32, name="y0Q")
            ymS = glob.tile([P, 2 * NB], F32, name="ymS")
            ymQ = glob.tile([P, 2 * NB], F32, name="ymQ")

            for pb in range(NB):
                ps0 = [pbps.tile([P, 512], F32, name=f"ps0_{ct}") for ct in range(2)]
                psm = [pbps.tile([P, 512], F32, name=f"psm_{ct}") for ct in range(2)]
                for k in range(KT):
                    xt = pbs.tile([P, 512], F32R, name="xt", bufs=4)
                    nc.sync.dma_start(
                        xt[:],
                        x_d[k * P:(k + 1) * P, pb * 512:(pb + 1) * 512].bitcast(F32R))
                    for ct in range(2):
                        nc.tensor.matmul(ps0[ct][:], w0t[:, k, ct * P:(ct + 1) * P],
                                         xt[:], start=(k == 0), stop=(k == KT - 1))
                        nc.tensor.matmul(psm[ct][:], wmt[:, k, ct * P:(ct + 1) * P],
                                         xt[:].bitcast(F32), start=(k == 0),
                                         stop=(k == KT - 1))
                    nc.vector.reduce_sum(pooled_cols[:, k * NB + pb:k * NB + pb + 1],
                                         xt[:].bitcast(F32), axis=AX.X)
                for ct in range(2):
                    st = pbs.tile([P, 512], F32, name="st0b", bufs=4)
                    nc.scalar.activation(st[:], ps0[ct][:], AF.Identity,
                                         accum_out=y0S[:, ct * NB + pb:ct * NB + pb + 1])
                    sq = pbs.tile([P, 512], F32, name="sq0b", bufs=4)
                    nc.scalar.activation(sq[:], ps0[ct][:], AF.Square,
                                         accum_out=y0Q[:, ct * NB + pb:ct * NB + pb + 1])
                    nc.sync.dma_start(
                        y0_dr[ct * P:(ct + 1) * P, pb * 512:(pb + 1) * 512], st[:])
                    stm = pbs.tile([P, 512], F32, name="stm0b", bufs=4)
                    nc.scalar.activation(stm[:], psm[ct][:], AF.Identity,
                                         accum_out=ymS[:, ct * NB + pb:ct * NB + pb + 1])
                    sqm = pbs.tile([P, 512], F32, name="sqm0b", bufs=4)
                    nc.scalar.activation(sqm[:], psm[ct][:], AF.Square,
                                         accum_out=ymQ[:, ct * NB + pb:ct * NB + pb + 1])
                    nc.sync.dma_start(
                        ymax_dr[ct * P:(ct + 1) * P, pb * 512:(pb + 1) * 512], stm[:])
            for ct in range(2):
                nc.vector.reduce_sum(stats[:, 12 + ct:13 + ct], y0S[:, ct * NB:(ct + 1) * NB], axis=AX.X)
                nc.vector.reduce_sum(stats[:, 14 + ct:15 + ct], y0Q[:, ct * NB:(ct + 1) * NB], axis=AX.X)
                nc.vector.reduce_sum(stats[:, 16 + ct:17 + ct], ymS[:, ct * NB:(ct + 1) * NB], axis=AX.X)
                nc.vector.reduce_sum(stats[:, 18 + ct:19 + ct], ymQ[:, ct * NB:(ct + 1) * NB], axis=AX.X)

            # pooled branch: y_pool = w_pool.T @ mean_pix(x)
            pooled = pbs.tile([P, KT], F32, name="pooled")
            for k in range(KT):
                nc.vector.reduce_sum(pooled[:, k:k + 1],
                                     pooled_cols[:, k * NB:(k + 1) * NB], axis=AX.X)
            nc.vector.tensor_scalar_mul(pooled[:], pooled[:], 1.0 / HW)
            for ct in range(2):
                pyp = pbps.tile([P, 1], F32, name=f"pyp_{ct}")
                for k in range(KT):
                    nc.tensor.matmul(pyp[:], wpt[:, k, ct * P:(ct + 1) * P],
                                     pooled[:, k:k + 1], start=(k == 0),
                                     stop=(k == KT - 1))
                nc.scalar.activation(stats[:, 22 + ct:23 + ct], pyp[:], AF.Identity)
                nc.scalar.activation(stats[:, 24 + ct:25 + ct], pyp[:], AF.Square)

        # ---------------- all-reduce of BN statistics ----------------
        nc.sync.dma_start(stats_in[:], stats[:])
        nc.gpsimd.collective_compute(
            "AllReduce", ALU.add, replica_groups=RG,
            ins=[stats_in.opt()], outs=[stats_out.opt()])
        nc.sync.dma_start(statsg[:], stats_out[:])

        # ---------------- phase 1: BN apply + xm + ch_sums + acc5 + low ----------------
        with (
            tc.tile_pool(name="p1", bufs=1) as p1,
            tc.tile_pool(name="p1_ps", bufs=1, space="PSUM") as p1ps,
        ):
            # scale/bias cols: instance ids
            #  0..5: dil br,ct ; 6,7: y0 ct ; 8,9: ymax ct ; 10: low ; 11,12: pool ct
            sbc = glob.tile([P, 26], F32, name="sbc")
            for br in range(3):
                for ct in range(2):
                    _bn_cols(nc, p1, statsg, sbc, br * 2 + ct,
                             br * 4 + ct, br * 4 + 2 + ct, N_HW)
            for ct in range(2):
                _bn_cols(nc, p1, statsg, sbc, 6 + ct, 12 + ct, 14 + ct, N_HW)
                _bn_cols(nc, p1, statsg, sbc, 8 + ct, 16 + ct, 18 + ct, N_HW)
            _bn_cols(nc, p1, statsg, sbc, 10, 20, 21, N_LOW, rows=48)
            for ct in range(2):
                _bn_cols(nc, p1, statsg, sbc, 11 + ct, 22 + ct, 24 + ct, N_POOL)

            def s_of(i):
                return sbc[:, 2 * i:2 * i + 1]

            def b_of(i):
                return sbc[:, 2 * i + 1:2 * i + 2]

            # xm = bn_relu(ymax) -> output + ch_sums partials
            chs = glob.tile([P, 2 * NB], F32, name="chs")
            for ct in range(2):
                for pb in range(NB):
                    yt = p1.tile([P, 512], F32, name="ymx_in", bufs=4)
                    nc.sync.dma_start(
                        yt[:], ymax_dr[ct * P:(ct + 1) * P, pb * 512:(pb + 1) * 512])
                    xmt = p1.tile([P, 512], F32, name="xm_t", bufs=4)
                    nc.scalar.activation(xmt[:], yt[:], AF.Relu,
                                         bias=b_of(8 + ct), scale=s_of(8 + ct),
                                         accum_out=chs[:, ct * NB + pb:ct * NB + pb + 1])
                    nc.sync.dma_start(
                        xm_o[ct * P:(ct + 1) * P, pb * 512:(pb + 1) * 512], xmt[:])
            chsc = p1.tile([P, 2], F32, name="chsc")
            for ct in range(2):
                nc.vector.reduce_sum(chsc[:, ct:ct + 1], chs[:, ct * NB:(ct + 1) * NB], axis=AX.X)
                nc.sync.dma_start(chs_o[ct * P:(ct + 1) * P, :], chsc[:, ct:ct + 1])

            # low output
            for pb in range(32):
                ylt = p1.tile([P, 512], F32, name="yl_in", bufs=4)
                nc.sync.dma_start(ylt[0:48, :], ylow_dr[:, pb * 512:(pb + 1) * 512])
                lot = p1.tile([P, 512], F32, name="lo_t", bufs=4)
                nc.scalar.activation(lot[0:48, :], ylt[0:48, :], AF.Relu,
                                     bias=b_of(10)[0:48, :], scale=s_of(10)[0:48, :])
                nc.sync.dma_start(low_o[:, pb * 512:(pb + 1) * 512], lot[0:48, :])

            # r4 vector and pvec = wproj4.T @ r4
            wpj = p1.tile([P, 10, 256], F32R, name="wpj")
            nc.sync.dma_start(
                wpj[:],
                wproj_d[0:1280, :].rearrange("(t p) c -> p t c", p=P).bitcast(F32R))
            # r4 is per-sample: this core's own y_pool (local stats col), with
            # batch statistics from the all-reduced copy. fp32r matmuls need an
            # even moving free count, so pair each r4 column with a zero col.
            r4p = p1.tile([P, 4], F32R, name="r4p")
            for ct in range(2):
                nc.scalar.activation(r4p[:, 2 * ct:2 * ct + 1],
                                     stats[:, 22 + ct:23 + ct],
                                     AF.Relu, bias=b_of(11 + ct), scale=s_of(11 + ct))
                # zero companion column (fp32r even-count padding)
                nc.scalar.activation(r4p[:, 2 * ct + 1:2 * ct + 2],
                                     stats[:, 22 + ct:23 + ct],
                                     AF.Identity, bias=0.0, scale=0.0)
            pvec = p1.tile([P, 2], F32, name="pvec")
            for cto in range(2):
                pps4 = p1ps.tile([P, 2], F32, name=f"pps4_{cto}")
                for cti in range(2):
                    nc.tensor.matmul(pps4[:], wpj[:, 8 + cti, cto * P:(cto + 1) * P],
                                     r4p[:, 2 * cti:2 * cti + 2], start=(cti == 0),
                                     stop=(cti == 1))
                nc.scalar.activation(pvec[:, cto:cto + 1], pps4[:, 0:1], AF.Identity)

            # acc5 = sum_{br in 0..4} wproj_br.T @ r_br   (r5 added in stage B)
            for pb in range(NB):
                rts = {}
                for br in range(4):
                    for cti in range(2):
                        yt = p1.tile([P, 512], F32, name="acc5_in", bufs=6)
                        if br == 0:
                            src = y0_dr[cti * P:(cti + 1) * P, pb * 512:(pb + 1) * 512]
                            inst = 6 + cti
                        else:
                            src = ydil[br - 1, cti * P:(cti + 1) * P,
                                       pb * 512:(pb + 1) * 512]
                            inst = (br - 1) * 2 + cti
                        nc.sync.dma_start(yt[:], src)
                        rt = p1.tile([P, 512], F32R, name="acc5_r", bufs=6)
                        nc.scalar.activation(rt[:], yt[:], AF.Relu,
                                             bias=b_of(inst), scale=s_of(inst))
                        rts[(br, cti)] = rt
                for cto in range(2):
                    psa = p1ps.tile([P, 512], F32, name=f"psa_{cto}")
                    n_mm = 8
                    i = 0
                    for br in range(4):
                        for cti in range(2):
                            nc.tensor.matmul(psa[:],
                                             wpj[:, br * 2 + cti, cto * P:(cto + 1) * P],
                                             rts[(br, cti)][:],
                                             start=(i == 0), stop=(i == n_mm - 1))
                            i += 1
                    ot = p1.tile([P, 512], F32, name="acc5_out", bufs=4)
                    nc.scalar.activation(ot[:], psa[:], AF.Identity,
                                         bias=pvec[:, cto:cto + 1])
                    nc.sync.dma_start(
                        acc5_o[cto * P:(cto + 1) * P, pb * 512:(pb + 1) * 512], ot[:])

        glob_cm.__exit__(None, None, None)
        dram_cm.__exit__(None, None, None)
    nc.compile()
    return nc


def build_stage_b():
    nc = bacc.Bacc("TRN2", target_bir_lowering=False, debug=False)

    xsel_d = nc.dram_tensor("xsel", [9, HW], F32, kind="ExternalInput")
    acc5_d = nc.dram_tensor("acc5", [256, HW], F32, kind="ExternalInput")
    low_d = nc.dram_tensor("low", [48, 16384], F32, kind="ExternalInput")
    wsel_d = nc.dram_tensor("wsel", [9, 256], F32, kind="ExternalInput")
    wproj5_d = nc.dram_tensor("wproj5", [256, 256], F32, kind="ExternalInput")
    wcls1_d = nc.dram_tensor("wcls1", [304, 9 * 256], F32, kind="ExternalInput")
    wcls2_d = nc.dram_tensor("wcls2", [256, 21], F32, kind="ExternalInput")
    bcls2_d = nc.dram_tensor("bcls2", [21, 1], F32, kind="ExternalInput")

    out_o = nc.dram_tensor("out_o", [21, 16384], F32, kind="ExternalOutput")

    KTC = (128, 128, 48)  # cls1 contraction partition tiles over 304 channels

    with tile.TileContext(nc) as tc:
        glob_cm = tc.tile_pool(name="globB", bufs=1)
        dram_cm = tc.tile_pool(name="drampB", bufs=1, space="DRAM")
        glob = glob_cm.__enter__()
        dram = dram_cm.__enter__()

        cat_dr = dram.tile([304, 128 * 130], F32, name="cat_dr")  # width-padded (zero col 0/129)
        ycls1_dr = dram.tile([256, 16384], F32, name="ycls1_dr")
        ar_bufs = []
        for i in range(3):
            ai = dram.tile([P, 4], F32, name=f"arin{i}")
            ao = dram.tile([P, 4], F32, name=f"arout{i}")
            ar_bufs.append((ai, ao))

        def allreduce4(idx, src, dst):
            ai, ao = ar_bufs[idx]
            nc.sync.dma_start(ai[:], src[:])
            nc.gpsimd.collective_compute(
                "AllReduce", ALU.add, replica_groups=RG,
                ins=[ai.opt()], outs=[ao.opt()])
            nc.sync.dma_start(dst[:], ao[:])

        def bn_sb(pool, statsg_t, tag, n):
            """statsg_t: [P,4] = (S0,S1,Q0,Q1) -> returns (s,b) cols tile [P,4]
            layout (s0,b0,s1,b1)."""
            sbt = glob.tile([P, 4], F32, name=f"sb_{tag}")
            for ct in range(2):
                m = pool.tile([P, 1], F32, name=f"m_{tag}{ct}")
                v = pool.tile([P, 1], F32, name=f"v_{tag}{ct}")
                t = pool.tile([P, 1], F32, name=f"t_{tag}{ct}")
                nc.vector.tensor_scalar_mul(m[:], statsg_t[:, ct:ct + 1], 1.0 / n)
                nc.vector.tensor_scalar_mul(v[:], statsg_t[:, 2 + ct:3 + ct], 1.0 / n)
                nc.vector.tensor_mul(t[:], m[:], m[:])
                nc.vector.tensor_sub(v[:], v[:], t[:])
                nc.vector.tensor_scalar_add(v[:], v[:], EPS)
                nc.vector.reciprocal(t[:], v[:])
                nc.scalar.activation(sbt[:, 2 * ct:2 * ct + 1], t[:], AF.Sqrt)
                nc.vector.tensor_mul(sbt[:, 2 * ct + 1:2 * ct + 2], m[:],
                                     sbt[:, 2 * ct:2 * ct + 1])
                nc.vector.tensor_scalar_mul(sbt[:, 2 * ct + 1:2 * ct + 2],
                                            sbt[:, 2 * ct + 1:2 * ct + 2], -1.0)
            return sbt

        # ---------------- y_sel + stats ----------------
        ysel_sb = [None, None]
        sel_stats = glob.tile([P, 4], F32, name="sel_stats")
        selS = glob.tile([P, 2 * NB], F32, name="selS")
        selQ = glob.tile([P, 2 * NB], F32, name="selQ")
        with (
            tc.tile_pool(name="psel_sb", bufs=1) as psb,
            tc.tile_pool(name="psel_ps", bufs=1, space="PSUM") as psps,
        ):
            wst = psb.tile([9, 256], F32, name="wst")
            nc.sync.dma_start(wst[:], wsel_d[:, :])
            for ct in range(2):
                ysel_sb[ct] = glob.tile([P, HW], F32, name=f"ysel{ct}")
            for pb in range(NB):
                xst = psb.tile([9, 512], F32, name="xst", bufs=3)
                nc.sync.dma_start(xst[:], xsel_d[:, pb * 512:(pb + 1) * 512])
                for ct in range(2):
                    pss = psps.tile([P, 512], F32, name=f"pss{ct}")
                    nc.tensor.matmul(pss[:], wst[:, ct * P:(ct + 1) * P], xst[:],
                                     start=True, stop=True)
                    nc.scalar.activation(ysel_sb[ct][:, pb * 512:(pb + 1) * 512],
                                         pss[:], AF.Identity,
                                         accum_out=selS[:, ct * NB + pb:ct * NB + pb + 1])
                    sq = psb.tile([P, 512], F32, name="sq_sel", bufs=2)
                    nc.scalar.activation(sq[:], pss[:], AF.Square,
                                         accum_out=selQ[:, ct * NB + pb:ct * NB + pb + 1])
            for ct in range(2):
                nc.vector.reduce_sum(sel_stats[:, ct:ct + 1], selS[:, ct * NB:(ct + 1) * NB], axis=AX.X)
                nc.vector.reduce_sum(sel_stats[:, 2 + ct:3 + ct],
                                     selQ[:, ct * NB:(ct + 1) * NB], axis=AX.X)
        sel_statsg = glob.tile([P, 4], F32, name="sel_statsg")
        allreduce4(0, sel_stats, sel_statsg)

        # ---------------- r5, proj5, y_aspp + stats ----------------
        yaspp_sb = [None, None]
        aspp_stats = glob.tile([P, 4], F32, name="aspp_stats")
        asS = glob.tile([P, 2 * NB], F32, name="asS")
        asQ = glob.tile([P, 2 * NB], F32, name="asQ")
        with (
            tc.tile_pool(name="pr5_sb", bufs=1) as pr5,
            tc.tile_pool(name="pr5_ps", bufs=1, space="PSUM") as pr5ps,
        ):
            sb_sel = bn_sb(pr5, sel_statsg, "sel", N_HW)
            r5 = [pr5.tile([P, HW], F32R, name=f"r5_{ct}") for ct in range(2)]
            for ct in range(2):
                nc.scalar.activation(r5[ct][:], ysel_sb[ct][:], AF.Relu,
                                     bias=sb_sel[:, 2 * ct + 1:2 * ct + 2],
                                     scale=sb_sel[:, 2 * ct:2 * ct + 1])
            wp5 = pr5.tile([P, 2, 256], F32R, name="wp5")
            nc.sync.dma_start(
                wp5[:],
                wproj5_d[:, :].rearrange("(t p) c -> p t c", p=P).bitcast(F32R))
            for ct in range(2):
                yaspp_sb[ct] = glob.tile([P, HW], F32, name=f"yaspp{ct}")
            for pb in range(NB):
                for cto in range(2):
                    ps5 = pr5ps.tile([P, 512], F32, name=f"ps5_{cto}")
                    for cti in range(2):
                        nc.tensor.matmul(ps5[:], wp5[:, cti, cto * P:(cto + 1) * P],
                                         r5[cti][:, pb * 512:(pb + 1) * 512],
                                         start=(cti == 0), stop=(cti == 1))
                    a5t = pr5.tile([P, 512], F32, name="a5t", bufs=4)
                    nc.sync.dma_start(
                        a5t[:], acc5_d[cto * P:(cto + 1) * P, pb * 512:(pb + 1) * 512])
                    dst = yaspp_sb[cto][:, pb * 512:(pb + 1) * 512]
                    nc.vector.scalar_tensor_tensor(
                        dst, ps5[:], 1.0, a5t[:], op0=ALU.mult, op1=ALU.add,
                        accum_out=asS[:, cto * NB + pb:cto * NB + pb + 1])
                    sq = pr5.tile([P, 512], F32, name="sq_as", bufs=2)
                    nc.scalar.activation(sq[:], dst, AF.Square,
                                         accum_out=asQ[:, cto * NB + pb:cto * NB + pb + 1])
            for ct in range(2):
                nc.vector.reduce_sum(aspp_stats[:, ct:ct + 1], asS[:, ct * NB:(ct + 1) * NB], axis=AX.X)
                nc.vector.reduce_sum(aspp_stats[:, 2 + ct:3 + ct],
                                     asQ[:, ct * NB:(ct + 1) * NB], axis=AX.X)
        aspp_statsg = glob.tile([P, 4], F32, name="aspp_statsg")
        allreduce4(1, aspp_stats, aspp_statsg)

        # ---------------- aspp, bilinear x2 upsample, cat assembly ----------------
        # zero the width-pad columns, then low -> cat rows 0:48
        cat4 = cat_dr.rearrange("p (h w) -> p h w", h=128, w=130)
        with tc.tile_pool(name="pz", bufs=1) as pz:
            zt = pz.tile([P, 304], F32, name="zt")
            nc.vector.memset(zt[:], 0.0)
            nc.sync.dma_start(cat4[:, :, 0:1], zt[:])
            nc.sync.dma_start(cat4[:, :, 129:130], zt[:])
        nc.sync.dma_start(
            cat4[0:48, :, 1:129],
            low_d[:, :].rearrange("p (h w) -> p h w", h=128, w=128))
        with tc.tile_pool(name="pup", bufs=1) as pup:
            sb_as = bn_sb(pup, aspp_statsg, "aspp", N_HW)
            for ct in range(2):
                aspp = pup.tile([P, HW], F32, name="aspp_t")
                nc.scalar.activation(aspp[:], yaspp_sb[ct][:], AF.Relu,
                                     bias=sb_as[:, 2 * ct + 1:2 * ct + 2],
                                     scale=sb_as[:, 2 * ct:2 * ct + 1])
                # H-pass: out rows interleave as (even, odd) pairs per source row
                #   out[2j]   = .75 src[j] + .25 src[j-1]   (j>=1; out[0]=src[0])
                #   out[2j+1] = .75 src[j] + .25 src[j+1]   (j<=62; out[127]=src[63])
                src_m = aspp.rearrange("p (h one w) -> p h one w", one=1, w=W64)
                s25 = pup.tile([P, HW], F32, name="s25")
                nc.vector.tensor_scalar_mul(s25[:], aspp[:], 0.25)
                s25_m = s25.rearrange("p (h one w) -> p h one w", one=1, w=W64)
                hm = pup.tile([P, 128 * W64], F32, name="hm")
                hme = hm.rearrange("p (h two w) -> p h two w", two=2, w=W64)
                nc.scalar.activation(hme[:, 0:1, 0:1, :], src_m[:, 0:1, :, :],
                                     AF.Identity)
                nc.vector.scalar_tensor_tensor(
                    hme[:, 1:W64, 0:1, :], src_m[:, 1:W64, :, :], 0.75,
                    s25_m[:, 0:W64 - 1, :, :], op0=ALU.mult, op1=ALU.add)
                nc.vector.scalar_tensor_tensor(
                    hme[:, 0:W64 - 1, 1:2, :], src_m[:, 0:W64 - 1, :, :], 0.75,
                    s25_m[:, 1:W64, :, :], op0=ALU.mult, op1=ALU.add)
                nc.scalar.activation(hme[:, W64 - 1:W64, 1:2, :],
                                     src_m[:, W64 - 1:W64, :, :], AF.Identity)
                # W-pass, chunked by 16 output rows
                s25w = pup.tile([P, 128 * W64], F32, name="s25w")
                nc.vector.tensor_scalar_mul(s25w[:], hm[:], 0.25)
                hm_m = hm.rearrange("p (h w one) -> p h w one", one=1, h=128, w=W64)
                sw_m = s25w.rearrange("p (h w one) -> p h w one", one=1, h=128, w=W64)
                for rc in range(8):
                    r0, r1 = rc * 16, rc * 16 + 16
                    wo = pup.tile([P, 16, 128], F32, name="wo", bufs=3)
                    woe = wo.rearrange("p h (w two) -> p h w two", two=2)
                    nc.scalar.activation(woe[:, :, 0:1, 0:1], hm_m[:, r0:r1, 0:1, :],
                                         AF.Identity)
                    nc.vector.scalar_tensor_tensor(
                        woe[:, :, 1:W64, 0:1], hm_m[:, r0:r1, 1:W64, :], 0.75,
                        sw_m[:, r0:r1, 0:W64 - 1, :], op0=ALU.mult, op1=ALU.add)
                    nc.vector.scalar_tensor_tensor(
                        woe[:, :, 0:W64 - 1, 1:2], hm_m[:, r0:r1, 0:W64 - 1, :], 0.75,
                        sw_m[:, r0:r1, 1:W64, :], op0=ALU.mult, op1=ALU.add)
                    nc.scalar.activation(woe[:, :, W64 - 1:W64, 1:2],
                                         hm_m[:, r0:r1, W64 - 1:W64, :], AF.Identity)
                    nc.sync.dma_start(
                        cat4[48 + ct * P:48 + (ct + 1) * P, r0:r1, 1:129],
                        wo[:])

        # ---------------- cls1: 3x3 conv, pad 1, over 304 channels ----------------
        c1_stats = glob.tile([P, 4], F32, name="c1_stats")
        c1S = glob.tile([P, 64], F32, name="c1S")
        c1Q = glob.tile([P, 64], F32, name="c1Q")
        with (
            tc.tile_pool(name="pc1", bufs=1) as pc1,
            tc.tile_pool(name="pc1_ps", bufs=1, space="PSUM") as pc1ps,
        ):
            wc1 = []
            roff = 0
            for kt, rows in enumerate(KTC):
                wt = pc1.tile([rows, 9, 256], F32R, name=f"wc1_{kt}")
                nc.sync.dma_start(
                    wt[:],
                    wcls1_d[roff:roff + rows, :]
                    .rearrange("p (t c) -> p t c", t=9).bitcast(F32R))
                wc1.append(wt)
                roff += rows
            for rb2 in range(8):
                yy0 = rb2 * 16
                slo = max(0, yy0 - 1)
                shi = min(128, yy0 + 17)
                nrows = shi - slo
                slabs = []
                roff = 0
                for kt, rows in enumerate(KTC):
                    sl = pc1.tile([rows, 18, 130], F32R, name=f"slab{kt}", bufs=2)
                    nc.sync.dma_start(
                        sl[:, 0:nrows, :],
                        cat4[roff:roff + rows, slo:shi, :].bitcast(F32R))
                    slabs.append(sl)
                    roff += rows
                for sub in range(4):
                    r0 = yy0 + sub * 4  # global output row
                    for cto in range(2):
                        psc = pc1ps.tile([P, 512], F32, name=f"psc{cto}")
                        ps3 = psc.rearrange("p (h w) -> p h w", h=4)
                        taps = []
                        for iy, dy in enumerate((-1, 0, 1)):
                            ylo = max(r0, -dy)
                            yhi = min(r0 + 4, 128 - dy)
                            if yhi <= ylo:
                                continue
                            for ix, dx in enumerate((-1, 0, 1)):
                                for kt in range(3):
                                    taps.append((iy * 3 + ix, kt, dy, dx, ylo, yhi))
                        nt = len(taps)
                        for i, (t, kt, dy, dx, ylo, yhi) in enumerate(taps):
                            sl3 = slabs[kt]
                            nc.tensor.matmul(
                                ps3[:, ylo - r0:yhi - r0, :],
                                wc1[kt][:, t, cto * P:(cto + 1) * P],
                                sl3[:, ylo + dy - slo:yhi + dy - slo,
                                    1 + dx:129 + dx],
                                start=(i == 0), stop=(i == nt - 1))
                        stc = pc1.tile([P, 512], F32, name="stc1", bufs=4)
                        nc.scalar.activation(stc[:], psc[:], AF.Identity,
                                             accum_out=c1S[:, cto * 32 + rb2 * 4 + sub:cto * 32 + rb2 * 4 + sub + 1])
                        sqc = pc1.tile([P, 512], F32, name="sqc1", bufs=4)
                        nc.scalar.activation(sqc[:], psc[:], AF.Square,
                                             accum_out=c1Q[:, cto * 32 + rb2 * 4 + sub:cto * 32 + rb2 * 4 + sub + 1])
                        nc.sync.dma_start(
                            ycls1_dr[cto * P:(cto + 1) * P, r0 * 128:(r0 + 4) * 128],
                            stc[:])
            for ct in range(2):
                nc.vector.reduce_sum(c1_stats[:, ct:ct + 1], c1S[:, ct * 32:(ct + 1) * 32], axis=AX.X)
                nc.vector.reduce_sum(c1_stats[:, 2 + ct:3 + ct],
                                     c1Q[:, ct * 32:(ct + 1) * 32], axis=AX.X)
        c1_statsg = glob.tile([P, 4], F32, name="c1_statsg")
        allreduce4(2, c1_stats, c1_statsg)

        # ---------------- h = bn_relu(ycls1); out = wcls2.T @ h + b ----------------
        with (
            tc.tile_pool(name="pc2", bufs=1) as pc2,
            tc.tile_pool(name="pc2_ps", bufs=1, space="PSUM") as pc2ps,
        ):
            sb_c1 = bn_sb(pc2, c1_statsg, "c1", N_LOW)
            wc2 = pc2.tile([P, 2, 21], F32R, name="wc2")
            nc.sync.dma_start(
                wc2[:], wcls2_d[:, :].rearrange("(t p) c -> p t c", p=P).bitcast(F32R))
            bct = pc2.tile([21, 1], F32, name="bct")
            nc.sync.dma_start(bct[:], bcls2_d[:, :])
            for pb in range(32):
                hts = []
                for kt in range(2):
                    yt = pc2.tile([P, 512], F32, name="yc1_in", bufs=4)
                    nc.sync.dma_start(
                        yt[:], ycls1_dr[kt * P:(kt + 1) * P, pb * 512:(pb + 1) * 512])
                    ht = pc2.tile([P, 512], F32R, name="h_t", bufs=4)
                    nc.scalar.activation(ht[:], yt[:], AF.Relu,
                                         bias=sb_c1[:, 2 * kt + 1:2 * kt + 2],
                                         scale=sb_c1[:, 2 * kt:2 * kt + 1])
                    hts.append(ht)
                ps2 = pc2ps.tile([P, 512], F32, name="ps2")
                for kt in range(2):
                    nc.tensor.matmul(ps2[0:21, :], wc2[:, kt, :], hts[kt][:],
                                     start=(kt == 0), stop=(kt == 1))
                ot = pc2.tile([P, 512], F32, name="out_t", bufs=4)
                nc.scalar.activation(ot[0:21, :], ps2[0:21, :], AF.Identity,
                                     bias=bct[:])
                nc.sync.dma_start(out_o[:, pb * 512:(pb + 1) * 512], ot[0:21, :])

        glob_cm.__exit__(None, None, None)
        dram_cm.__exit__(None, None, None)
    nc.compile()
    return nc


# --------------------------------------------------------------------------
# host-side PSO (bit-replica of the reference, jax on CPU)
# --------------------------------------------------------------------------

def _pso_select_host(ch_sums, init_particles, rands):
    import jax
    import jax.numpy as jnp
    from jax import lax

    INERTIA, COG, SOC = 0.7, 1.5, 1.5
    num_filters = 256

    def pso(channel_sums, init_p, rs):
        particles = init_p.astype(jnp.float32)
        velocities = jnp.zeros_like(particles)

        def obj(p):
            return channel_sums[jnp.floor(p).astype(jnp.int32)].sum()

        best_pos = particles
        best_scores = jax.vmap(obj)(particles)
        gi = jnp.argmin(best_scores)
        g_pos, g_score = particles[gi], best_scores[gi]
        n_particles = particles.shape[0]

        def iter_step(carry, rand_it):
            def particle_step(c, i):
                particles, velocities, best_pos, best_scores, g_pos, g_score = c
                r1, r2 = rand_it[i, 0], rand_it[i, 1]
                v = (INERTIA * velocities[i]
                     + COG * r1 * (best_pos[i] - particles[i])
                     + SOC * r2 * (g_pos - particles[i]))
                p = jnp.clip(particles[i] + v, 0.0, num_filters - 1.0)
                fit = obj(p)
                better = fit < best_scores[i]
                best_pos = best_pos.at[i].set(jnp.where(better, p, best_pos[i]))
                best_scores = best_scores.at[i].set(
                    jnp.where(better, fit, best_scores[i]))
                gbetter = fit < g_score
                g_pos = jnp.where(gbetter, p, g_pos)
                g_score = jnp.where(gbetter, fit, g_score)
                particles = particles.at[i].set(p)
                velocities = velocities.at[i].set(v)
                return (particles, velocities, best_pos, best_scores, g_pos,
                        g_score), None
            c, _ = lax.scan(particle_step, carry, jnp.arange(n_particles))
            return c, None

        carry = (particles, velocities, best_pos, best_scores, g_pos, g_score)
        carry, _ = lax.scan(iter_step, carry, rs)
        return jnp.floor(carry[4]).astype(jnp.int32)

    cpu = jax.devices("cpu")[0]
    with jax.default_device(cpu):
        best = jax.jit(pso)(jnp.asarray(ch_sums), jnp.asarray(init_particles),
                            jnp.asarray(rands))
        return np.asarray(best)


# --------------------------------------------------------------------------
# orchestration
# --------------------------------------------------------------------------

_NC_CACHE = {}


def _get_nc(stage):
    if stage not in _NC_CACHE:
        _NC_CACHE[stage] = build_stage_a() if stage == "a" else build_stage_b()
    return _NC_CACHE[stage]


def _run(nc, in_maps):
    res = run_bass_kernel_spmd(nc, in_maps, core_ids=list(range(NCORES)),
                               trace=_trace)
    return res


def kernel(feature_out, feature_low, w_low, w_a0, w_a1, w_a2, w_a3, w_pool,
           w_max, w_sel, w_proj, w_cls1, w_cls2, b_cls2, rand_scalars,
           init_particles):
    feature_out = np.ascontiguousarray(np.asarray(feature_out, dtype=np.float32))
    feature_low = np.ascontiguousarray(np.asarray(feature_low, dtype=np.float32))

    def t2d(w):  # [cout, cin, 1, 1] -> [cin, cout]
        return np.ascontiguousarray(
            np.asarray(w, np.float32).reshape(w.shape[0], w.shape[1]).T)

    wdil = np.stack([np.asarray(w, np.float32) for w in (w_a1, w_a2, w_a3)])
    # [3, 256, 2048, 3, 3] -> [2048, 3, 9, 256] -> [2048, 3*9*256]
    wdil = np.ascontiguousarray(
        wdil.transpose(2, 0, 3, 4, 1).reshape(2048, 3, 9, 256).reshape(2048, -1))
    wa0 = t2d(w_a0)
    wmax = t2d(w_max)
    wpool = t2d(w_pool)
    wlow = t2d(w_low)
    wproj = t2d(w_proj)                       # [1536, 256]
    wsel = t2d(w_sel)                         # [9, 256]
    # [256, 304, 3, 3] -> [304, 3, 3, 256] -> [304, 9*256]
    wcls1 = np.ascontiguousarray(
        np.asarray(w_cls1, np.float32).transpose(1, 2, 3, 0).reshape(304, -1))
    wcls2 = t2d(w_cls2)                       # [256, 21]
    bcls2 = np.ascontiguousarray(np.asarray(b_cls2, np.float32).reshape(21, 1))

    nc_a = _get_nc("a")
    in_maps_a = []
    for i in range(NCORES):
        in_maps_a.append({
            "x": feature_out[i].reshape(2048, HW),
            "wdil": wdil, "wa0": wa0, "wmax": wmax, "wpool": wpool,
            "xlow": feature_low[i].reshape(256, 16384),
            "wlow": wlow, "wproj": wproj,
        })
    res_a = _run(nc_a, in_maps_a)
    if _trace:
        LAST_EXEC_NS["a"] = res_a.exec_time_ns

    xm = np.stack([res_a.results[i]["xm_o"] for i in range(NCORES)])   # [8,256,4096]
    chs = np.sum([res_a.results[i]["chs_o"][:, 0] for i in range(NCORES)], axis=0)
    best = _pso_select_host(chs.astype(np.float32),
                            np.asarray(init_particles, np.int32),
                            np.asarray(rand_scalars, np.float32))
    xsel = np.ascontiguousarray(xm[:, best, :])                        # [8,9,4096]

    nc_b = _get_nc("b")
    in_maps_b = []
    for i in range(NCORES):
        in_maps_b.append({
            "xsel": xsel[i],
            "acc5": res_a.results[i]["acc5_o"],
            "low": res_a.results[i]["low_o"],
            "wsel": wsel, "wproj5": np.ascontiguousarray(wproj[1280:1536]),
            "wcls1": wcls1, "wcls2": wcls2, "bcls2": bcls2,
        })
    res_b = _run(nc_b, in_maps_b)
    if _trace:
        LAST_EXEC_NS["b"] = res_b.exec_time_ns

    out = np.stack([res_b.results[i]["out_o"].reshape(21, 128, 128)
                    for i in range(NCORES)])
    return out.astype(np.float32)


# revision 17
# speedup vs baseline: 1.1869x; 1.1869x over previous
"""Trainium2 Bass kernel for nn_DeepLabHeadV3Plus (8-core SPMD, batch-sharded).

Strategy
--------
Data-parallel over batch: core i owns sample i (B=8, 8 cores). Training-mode
BatchNorm statistics are all-reduced across cores. The PSO channel-selection
loop (inherently sequential, 900 tiny steps) runs on the host between two
device stages, replicated bit-for-bit with jax-on-CPU:

  Stage A (device): low-projection conv, ASPP branch convs (1x1, three dilated
      3x3, pooled 1x1, w_max 1x1), BN stats all-reduce, BN+ReLU, xm,
      per-channel partial sums for PSO, and the r0..r4 part of the 1x1
      projection (acc5).
  Host: global ch_sums, PSO -> best[9], x_sel gather from xm.
  Stage B (device): r5 conv + proj, aspp BN (AR), bilinear x2 upsample,
      concat with low, 3x3 classifier conv + BN (AR), final 1x1 conv + bias.

Matmuls run as float32r (fp32 with hardware RNE rounding to 11 mantissa bits
at the PE input, full-rate for N>=256). The w_max conv stays in exact fp32 so
the discrete PSO selection is numerically stable.
"""
import os
import sys

for _p in ("/opt/trn_rl_repo", "/root/.axon_site/_ro/trn_rl_repo"):
    if os.path.isdir(_p) and _p not in sys.path:
        sys.path.insert(0, _p)
        break

import numpy as np

import concourse.bass as bass  # noqa: F401  (engine types via nc)
import concourse.mybir as mybir
import concourse.tile as tile
from concourse import bacc
from concourse.bass_utils import run_bass_kernel_spmd

F32 = mybir.dt.float32
F32R = mybir.dt.float32r
AF = mybir.ActivationFunctionType
ALU = mybir.AluOpType
AX = mybir.AxisListType

P = 128
NCORES = 8
HW = 4096            # 64*64
W64 = 64
KT = 16              # 2048 / 128 cin tiles
NB = 8               # pixel blocks of 512 over 64x64
DILS = (12, 24, 36)
EPS = 1e-5
N_HW = 8 * HW        # BN count for 64x64 maps (batch*spatial)
N_LOW = 8 * 16384    # BN count for 128x128 maps
N_POOL = 8           # BN count for pooled branch

RG = [list(range(NCORES))]

_trace = bool(int(os.environ.get("TRN_TRACE", "0")))
LAST_EXEC_NS = {}


# --------------------------------------------------------------------------
# builder helpers
# --------------------------------------------------------------------------

def _bn_cols(nc, pool, statsg, s_cols, i_inst, cS, cQ, n, rows=P):
    """From global-sum cols statsg[:,cS] (sum) and statsg[:,cQ] (sum of sq),
    write scale into s_cols[:, 2*i] and bias into s_cols[:, 2*i+1]:
      s = 1/sqrt(var+eps), b = -mean*s."""
    m = pool.tile([P, 1], F32, name=f"bn_m_{i_inst}")
    v = pool.tile([P, 1], F32, name=f"bn_v_{i_inst}")
    t = pool.tile([P, 1], F32, name=f"bn_t_{i_inst}")
    r = slice(0, rows)
    sc = s_cols[r, 2 * i_inst:2 * i_inst + 1]
    bc = s_cols[r, 2 * i_inst + 1:2 * i_inst + 2]
    nc.vector.tensor_scalar_mul(m[r, :], statsg[r, cS:cS + 1], 1.0 / n)
    nc.vector.tensor_scalar_mul(v[r, :], statsg[r, cQ:cQ + 1], 1.0 / n)
    nc.vector.tensor_mul(t[r, :], m[r, :], m[r, :])
    nc.vector.tensor_sub(v[r, :], v[r, :], t[r, :])
    nc.vector.tensor_scalar_add(v[r, :], v[r, :], EPS)
    nc.vector.reciprocal(t[r, :], v[r, :])
    nc.scalar.activation(sc, t[r, :], AF.Sqrt)
    nc.vector.tensor_mul(bc, m[r, :], sc)
    nc.vector.tensor_scalar_mul(bc, bc, -1.0)


def build_stage_a():
    nc = bacc.Bacc("TRN2", target_bir_lowering=False, debug=False)

    x_d = nc.dram_tensor("x", [2048, HW], F32, kind="ExternalInput")
    wdil_d = nc.dram_tensor("wdil", [2048, 3 * 9 * 256], F32, kind="ExternalInput")
    wa0_d = nc.dram_tensor("wa0", [2048, 256], F32, kind="ExternalInput")
    wmax_d = nc.dram_tensor("wmax", [2048, 256], F32, kind="ExternalInput")
    wpool_d = nc.dram_tensor("wpool", [2048, 256], F32, kind="ExternalInput")
    xlow_d = nc.dram_tensor("xlow", [256, 16384], F32, kind="ExternalInput")
    wlow_d = nc.dram_tensor("wlow", [256, 48], F32, kind="ExternalInput")
    wproj_d = nc.dram_tensor("wproj", [1536, 256], F32, kind="ExternalInput")

    xm_o = nc.dram_tensor("xm_o", [256, HW], F32, kind="ExternalOutput")
    chs_o = nc.dram_tensor("chs_o", [256, 1], F32, kind="ExternalOutput")
    acc5_o = nc.dram_tensor("acc5_o", [256, HW], F32, kind="ExternalOutput")
    low_o = nc.dram_tensor("low_o", [48, 16384], F32, kind="ExternalOutput")

    with tile.TileContext(nc) as tc:
        glob_cm = tc.tile_pool(name="glob", bufs=1)
        dram_cm = tc.tile_pool(name="dramp", bufs=1, space="DRAM")
        glob = glob_cm.__enter__()
        dram = dram_cm.__enter__()

        ydil = dram.tile([3, 256, HW], F32, name="ydil")
        y0_dr = dram.tile([256, HW], F32, name="y0_dr")
        ymax_dr = dram.tile([256, HW], F32, name="ymax_dr")
        ylow_dr = dram.tile([48, 16384], F32, name="ylow_dr")
        stats_in = dram.tile([P, 26], F32, name="stats_in")
        stats_out = dram.tile([P, 26], F32, name="stats_out")

        # stat columns (local sums), packed for one all-reduce
        # 0..11  : dilated   [br*4 + (S0,S1,Q0,Q1)]
        # 12..15 : y0        (S0,S1,Q0,Q1)
        # 16..19 : ymax
        # 20,21  : low       (S,Q) rows 0:48
        # 22..25 : ypool     (v0,v1,q0,q1)
        stats = glob.tile([P, 26], F32, name="stats")
        statsg = glob.tile([P, 26], F32, name="statsg")
        pooled_cols = glob.tile([P, KT * NB], F32, name="pooled_cols")

        # ---------------- phase 0c: low-level projection conv ----------------
        with (
            tc.tile_pool(name="p0c", bufs=1) as pc,
            tc.tile_pool(name="p0c_ps", bufs=1, space="PSUM") as pcps,
        ):
            wlt = pc.tile([P, 2, 48], F32R, name="wlt")
            nc.sync.dma_start(
                wlt[:], wlow_d[:, :].rearrange("(t p) c -> p t c", p=P).bitcast(F32R))
            lowS = glob.tile([P, 32], F32, name="lowS")
            lowQ = glob.tile([P, 32], F32, name="lowQ")
            for pb in range(32):
                pslow = pcps.tile([P, 512], F32, name="pslow")
                for k in range(2):
                    xlt = pc.tile([P, 512], F32R, name="xlt", bufs=3)
                    nc.sync.dma_start(
                        xlt[:],
                        xlow_d[k * P:(k + 1) * P, pb * 512:(pb + 1) * 512].bitcast(F32R))
                    nc.tensor.matmul(pslow[0:48, :], wlt[:, k, :], xlt[:],
                                     start=(k == 0), stop=(k == 1))
                st = pc.tile([P, 512], F32, name="st_low", bufs=2)
                nc.scalar.activation(st[0:48, :], pslow[0:48, :], AF.Identity,
                                     accum_out=lowS[0:48, pb:pb + 1])
                sq = pc.tile([P, 512], F32, name="sq_low", bufs=2)
                nc.scalar.activation(sq[0:48, :], pslow[0:48, :], AF.Square,
                                     accum_out=lowQ[0:48, pb:pb + 1])
                nc.sync.dma_start(ylow_dr[:, pb * 512:(pb + 1) * 512], st[0:48, :])
            nc.vector.reduce_sum(stats[0:48, 20:21], lowS[0:48, :], axis=AX.X)
            nc.vector.reduce_sum(stats[0:48, 21:22], lowQ[0:48, :], axis=AX.X)

        # ---------------- phase 0a: dilated 3x3 convs (k-outer) ----------------
        with (
            tc.tile_pool(name="p0a_acc", bufs=1) as pacc,
            tc.tile_pool(name="p0a_s", bufs=1) as ps_,
            tc.tile_pool(name="p0a_ps", bufs=1, space="PSUM") as pps,
        ):
            accs = {}
            for br in range(3):
                for ct in range(2):
                    accs[(br, ct)] = pacc.tile([P, HW], F32, name=f"acc{br}{ct}")
            for k in range(KT):
                wk = ps_.tile([P, 27 * 256], F32R, name="wk", bufs=2)
                nc.sync.dma_start(
                    wk[:], wdil_d[k * P:(k + 1) * P, :].bitcast(F32R))
                wk4 = wk.rearrange("p (b t c) -> p b t c", b=3, t=9)
                xk = ps_.tile([P, HW], F32R, name="xk", bufs=2)
                nc.sync.dma_start(xk[:], x_d[k * P:(k + 1) * P, :].bitcast(F32R))
                xk3 = xk.rearrange("p (h w) -> p h w", h=W64)
                for rb in range(8):
                    y0r = rb * 8
                    for br, d in enumerate(DILS):
                        for ct in range(2):
                            ps = pps.tile([P, 512], F32, name=f"ps{br}{ct}")
                            ps3 = ps.rearrange("p (h w) -> p h w", h=8)
                            taps = []
                            for iy, dy in enumerate((-d, 0, d)):
                                ylo = max(y0r, -dy)
                                yhi = min(y0r + 8, W64 - dy)
                                if yhi <= ylo:
                                    continue
                                for ix, dx in enumerate((-d, 0, d)):
                                    xlo = max(0, -dx)
                                    xhi = min(W64, W64 - dx)
                                    taps.append((iy * 3 + ix, dy, dx, ylo, yhi, xlo, xhi))
                            nt = len(taps)
                            for i, (t, dy, dx, ylo, yhi, xlo, xhi) in enumerate(taps):
                                nc.tensor.matmul(
                                    ps3[:, ylo - y0r:yhi - y0r, xlo:xhi],
                                    wk4[:, br, t, ct * P:(ct + 1) * P],
                                    xk3[:, ylo + dy:yhi + dy, xlo + dx:xhi + dx],
                                    start=(i == 0), stop=(i == nt - 1))
                            dst = accs[(br, ct)][:, y0r * W64:(y0r + 8) * W64]
                            if k == 0:
                                nc.scalar.activation(dst, ps[:], AF.Identity)
                            else:
                                nc.vector.tensor_add(dst, dst, ps[:])
            # stats + spill y to DRAM
            for br in range(3):
                for ct in range(2):
                    a = accs[(br, ct)]
                    st = ps_.tile([P, HW], F32, name="sq_dil", bufs=1)
                    nc.scalar.activation(st[:], a[:], AF.Square,
                                         accum_out=stats[:, br * 4 + 2 + ct:br * 4 + 3 + ct])
                    nc.vector.reduce_sum(stats[:, br * 4 + ct:br * 4 + 1 + ct],
                                         a[:], axis=AX.X)
                    nc.sync.dma_start(ydil[br, ct * P:(ct + 1) * P, :], a[:])

        # ---------------- phase 0b: 1x1 convs (w_a0 f32r, w_max fp32) ----------------
        with (
            tc.tile_pool(name="p0b_w", bufs=1) as pw,
            tc.tile_pool(name="p0b_s", bufs=1) as pbs,
            tc.tile_pool(name="p0b_ps", bufs=1, space="PSUM") as pbps,
        ):
            w0t = pw.tile([P, KT, 256], F32R, name="w0t")
            nc.sync.dma_start(
                w0t[:], wa0_d[:, :].rearrange("(t p) c -> p t c", p=P).bitcast(F32R))
            wmt = pw.tile([P, KT, 256], F32, name="wmt")
            nc.sync.dma_start(
                wmt[:], wmax_d[:, :].rearrange("(t p) c -> p t c", p=P))
            wpt = pw.tile([P, KT, 256], F32, name="wpt")
            nc.sync.dma_start(
                wpt[:], wpool_d[:, :].rearrange("(t p) c -> p t c", p=P))

            y0S = glob.tile([P, 2 * NB], F32, name="y0S")
            y0Q = glob.tile([P, 2 * NB], F32, name="y0Q")
            ymS = glob.tile([P, 2 * NB], F32, name="ymS")
            ymQ = glob.tile([P, 2 * NB], F32, name="ymQ")

            for pb in range(NB):
                ps0 = [pbps.tile([P, 512], F32, name=f"ps0_{ct}") for ct in range(2)]
                psm = [pbps.tile([P, 512], F32, name=f"psm_{ct}") for ct in range(2)]
                for k in range(KT):
                    xt = pbs.tile([P, 512], F32R, name="xt", bufs=4)
                    nc.sync.dma_start(
                        xt[:],
                        x_d[k * P:(k + 1) * P, pb * 512:(pb + 1) * 512].bitcast(F32R))
                    for ct in range(2):
                        nc.tensor.matmul(ps0[ct][:], w0t[:, k, ct * P:(ct + 1) * P],
                                         xt[:], start=(k == 0), stop=(k == KT - 1))
                        nc.tensor.matmul(psm[ct][:], wmt[:, k, ct * P:(ct + 1) * P],
                                         xt[:].bitcast(F32), start=(k == 0),
                                         stop=(k == KT - 1))
                    nc.vector.reduce_sum(pooled_cols[:, k * NB + pb:k * NB + pb + 1],
                                         xt[:].bitcast(F32), axis=AX.X)
                for ct in range(2):
                    st = pbs.tile([P, 512], F32, name="st0b", bufs=4)
                    nc.scalar.activation(st[:], ps0[ct][:], AF.Identity,
                                         accum_out=y0S[:, ct * NB + pb:ct * NB + pb + 1])
                    sq = pbs.tile([P, 512], F32, name="sq0b", bufs=4)
                    nc.scalar.activation(sq[:], ps0[ct][:], AF.Square,
                                         accum_out=y0Q[:, ct * NB + pb:ct * NB + pb + 1])
                    nc.sync.dma_start(
                        y0_dr[ct * P:(ct + 1) * P, pb * 512:(pb + 1) * 512], st[:])
                    stm = pbs.tile([P, 512], F32, name="stm0b", bufs=4)
                    nc.scalar.activation(stm[:], psm[ct][:], AF.Identity,
                                         accum_out=ymS[:, ct * NB + pb:ct * NB + pb + 1])
                    sqm = pbs.tile([P, 512], F32, name="sqm0b", bufs=4)
                    nc.scalar.activation(sqm[:], psm[ct][:], AF.Square,
                                         accum_out=ymQ[:, ct * NB + pb:ct * NB + pb + 1])
                    nc.sync.dma_start(
                        ymax_dr[ct * P:(ct + 1) * P, pb * 512:(pb + 1) * 512], stm[:])
            for ct in range(2):
                nc.vector.reduce_sum(stats[:, 12 + ct:13 + ct], y0S[:, ct * NB:(ct + 1) * NB], axis=AX.X)
                nc.vector.reduce_sum(stats[:, 14 + ct:15 + ct], y0Q[:, ct * NB:(ct + 1) * NB], axis=AX.X)
                nc.vector.reduce_sum(stats[:, 16 + ct:17 + ct], ymS[:, ct * NB:(ct + 1) * NB], axis=AX.X)
                nc.vector.reduce_sum(stats[:, 18 + ct:19 + ct], ymQ[:, ct * NB:(ct + 1) * NB], axis=AX.X)

            # pooled branch: y_pool = w_pool.T @ mean_pix(x)
            pooled = pbs.tile([P, KT], F32, name="pooled")
            for k in range(KT):
                nc.vector.reduce_sum(pooled[:, k:k + 1],
                                     pooled_cols[:, k * NB:(k + 1) * NB], axis=AX.X)
            nc.vector.tensor_scalar_mul(pooled[:], pooled[:], 1.0 / HW)
            for ct in range(2):
                pyp = pbps.tile([P, 1], F32, name=f"pyp_{ct}")
                for k in range(KT):
                    nc.tensor.matmul(pyp[:], wpt[:, k, ct * P:(ct + 1) * P],
                                     pooled[:, k:k + 1], start=(k == 0),
                                     stop=(k == KT - 1))
                nc.scalar.activation(stats[:, 22 + ct:23 + ct], pyp[:], AF.Identity)
                nc.scalar.activation(stats[:, 24 + ct:25 + ct], pyp[:], AF.Square)

        # ---------------- all-reduce of BN statistics ----------------
        nc.sync.dma_start(stats_in[:], stats[:])
        nc.gpsimd.collective_compute(
            "AllReduce", ALU.add, replica_groups=RG,
            ins=[stats_in.opt()], outs=[stats_out.opt()])
        nc.sync.dma_start(statsg[:], stats_out[:])

        # ---------------- phase 1: BN apply + xm + ch_sums + acc5 + low ----------------
        with (
            tc.tile_pool(name="p1", bufs=1) as p1,
            tc.tile_pool(name="p1_ps", bufs=1, space="PSUM") as p1ps,
        ):
            # scale/bias cols: instance ids
            #  0..5: dil br,ct ; 6,7: y0 ct ; 8,9: ymax ct ; 10: low ; 11,12: pool ct
            sbc = glob.tile([P, 26], F32, name="sbc")
            for br in range(3):
                for ct in range(2):
                    _bn_cols(nc, p1, statsg, sbc, br * 2 + ct,
                             br * 4 + ct, br * 4 + 2 + ct, N_HW)
            for ct in range(2):
                _bn_cols(nc, p1, statsg, sbc, 6 + ct, 12 + ct, 14 + ct, N_HW)
                _bn_cols(nc, p1, statsg, sbc, 8 + ct, 16 + ct, 18 + ct, N_HW)
            _bn_cols(nc, p1, statsg, sbc, 10, 20, 21, N_LOW, rows=48)
            for ct in range(2):
                _bn_cols(nc, p1, statsg, sbc, 11 + ct, 22 + ct, 24 + ct, N_POOL)

            def s_of(i):
                return sbc[:, 2 * i:2 * i + 1]

            def b_of(i):
                return sbc[:, 2 * i + 1:2 * i + 2]

            # xm = bn_relu(ymax) -> output + ch_sums partials
            chs = glob.tile([P, 2 * NB], F32, name="chs")
            for ct in range(2):
                for pb in range(NB):
                    yt = p1.tile([P, 512], F32, name="ymx_in", bufs=4)
                    nc.sync.dma_start(
                        yt[:], ymax_dr[ct * P:(ct + 1) * P, pb * 512:(pb + 1) * 512])
                    xmt = p1.tile([P, 512], F32, name="xm_t", bufs=4)
                    nc.scalar.activation(xmt[:], yt[:], AF.Relu,
                                         bias=b_of(8 + ct), scale=s_of(8 + ct),
                                         accum_out=chs[:, ct * NB + pb:ct * NB + pb + 1])
                    nc.sync.dma_start(
                        xm_o[ct * P:(ct + 1) * P, pb * 512:(pb + 1) * 512], xmt[:])
            chsc = p1.tile([P, 2], F32, name="chsc")
            for ct in range(2):
                nc.vector.reduce_sum(chsc[:, ct:ct + 1], chs[:, ct * NB:(ct + 1) * NB], axis=AX.X)
                nc.sync.dma_start(chs_o[ct * P:(ct + 1) * P, :], chsc[:, ct:ct + 1])

            # low output
            for pb in range(32):
                ylt = p1.tile([P, 512], F32, name="yl_in", bufs=4)
                nc.sync.dma_start(ylt[0:48, :], ylow_dr[:, pb * 512:(pb + 1) * 512])
                lot = p1.tile([P, 512], F32, name="lo_t", bufs=4)
                nc.scalar.activation(lot[0:48, :], ylt[0:48, :], AF.Relu,
                                     bias=b_of(10)[0:48, :], scale=s_of(10)[0:48, :])
                nc.sync.dma_start(low_o[:, pb * 512:(pb + 1) * 512], lot[0:48, :])

            # r4 vector and pvec = wproj4.T @ r4
            wpj = p1.tile([P, 10, 256], F32R, name="wpj")
            nc.sync.dma_start(
                wpj[:],
                wproj_d[0:1280, :].rearrange("(t p) c -> p t c", p=P).bitcast(F32R))
            # r4 is per-sample: this core's own y_pool (local stats col), with
            # batch statistics from the all-reduced copy. fp32r matmuls need an
            # even moving free count, so pair each r4 column with a zero col.
            r4p = p1.tile([P, 4], F32R, name="r4p")
            for ct in range(2):
                nc.scalar.activation(r4p[:, 2 * ct:2 * ct + 1],
                                     stats[:, 22 + ct:23 + ct],
                                     AF.Relu, bias=b_of(11 + ct), scale=s_of(11 + ct))
                # zero companion column (fp32r even-count padding)
                nc.scalar.activation(r4p[:, 2 * ct + 1:2 * ct + 2],
                                     stats[:, 22 + ct:23 + ct],
                                     AF.Identity, bias=0.0, scale=0.0)
            pvec = p1.tile([P, 2], F32, name="pvec")
            for cto in range(2):
                pps4 = p1ps.tile([P, 2], F32, name=f"pps4_{cto}")
                for cti in range(2):
                    nc.tensor.matmul(pps4[:], wpj[:, 8 + cti, cto * P:(cto + 1) * P],
                                     r4p[:, 2 * cti:2 * cti + 2], start=(cti == 0),
                                     stop=(cti == 1))
                nc.scalar.activation(pvec[:, cto:cto + 1], pps4[:, 0:1], AF.Identity)

            # acc5 = sum_{br in 0..4} wproj_br.T @ r_br   (r5 added in stage B)
            for pb in range(NB):
                rts = {}
                for br in range(4):
                    for cti in range(2):
                        yt = p1.tile([P, 512], F32, name="acc5_in", bufs=6)
                        if br == 0:
                            src = y0_dr[cti * P:(cti + 1) * P, pb * 512:(pb + 1) * 512]
                            inst = 6 + cti
                        else:
                            src = ydil[br - 1, cti * P:(cti + 1) * P,
                                       pb * 512:(pb + 1) * 512]
                            inst = (br - 1) * 2 + cti
                        nc.sync.dma_start(yt[:], src)
                        rt = p1.tile([P, 512], F32R, name="acc5_r", bufs=6)
                        nc.scalar.activation(rt[:], yt[:], AF.Relu,
                                             bias=b_of(inst), scale=s_of(inst))
                        rts[(br, cti)] = rt
                for cto in range(2):
                    psa = p1ps.tile([P, 512], F32, name=f"psa_{cto}")
                    n_mm = 8
                    i = 0
                    for br in range(4):
                        for cti in range(2):
                            nc.tensor.matmul(psa[:],
                                             wpj[:, br * 2 + cti, cto * P:(cto + 1) * P],
                                             rts[(br, cti)][:],
                                             start=(i == 0), stop=(i == n_mm - 1))
                            i += 1
                    ot = p1.tile([P, 512], F32, name="acc5_out", bufs=4)
                    nc.scalar.activation(ot[:], psa[:], AF.Identity,
                                         bias=pvec[:, cto:cto + 1])
                    nc.sync.dma_start(
                        acc5_o[cto * P:(cto + 1) * P, pb * 512:(pb + 1) * 512], ot[:])

        glob_cm.__exit__(None, None, None)
        dram_cm.__exit__(None, None, None)
    nc.compile()
    return nc


def build_stage_b():
    """r5 and the sel/aspp BN constants come from the host; only the cls1 BN
    needs an on-device all-reduce."""
    nc = bacc.Bacc("TRN2", target_bir_lowering=False, debug=False)

    r5_d = nc.dram_tensor("r5", [256, HW], F32, kind="ExternalInput")
    acc5_d = nc.dram_tensor("acc5", [256, HW], F32, kind="ExternalInput")
    lowp_d = nc.dram_tensor("lowp", [48, 128 * 130], F32, kind="ExternalInput")
    sbas_d = nc.dram_tensor("sbas", [P, 4], F32, kind="ExternalInput")
    wproj5_d = nc.dram_tensor("wproj5", [256, 256], F32, kind="ExternalInput")
    wcls1_d = nc.dram_tensor("wcls1", [304, 9 * 256], F32, kind="ExternalInput")
    wcls2_d = nc.dram_tensor("wcls2", [256, 21], F32, kind="ExternalInput")
    bcls2_d = nc.dram_tensor("bcls2", [21, 1], F32, kind="ExternalInput")

    out_o = nc.dram_tensor("out_o", [21, 16384], F32, kind="ExternalOutput")

    KTC = (128, 128, 48)  # cls1 contraction partition tiles over 304 channels

    with tile.TileContext(nc) as tc:
        glob_cm = tc.tile_pool(name="globB", bufs=1)
        dram_cm = tc.tile_pool(name="drampB", bufs=1, space="DRAM")
        glob = glob_cm.__enter__()
        dram = dram_cm.__enter__()

        up_dr = dram.tile([256, 128 * 130], F32, name="up_dr")  # width-padded
        ycls1_dr = dram.tile([256, 16384], F32, name="ycls1_dr")
        ar_in = dram.tile([P, 4], F32, name="ar_in")
        ar_out = dram.tile([P, 4], F32, name="ar_out")

        sbas = glob.tile([P, 4], F32, name="sbas_t")
        nc.sync.dma_start(sbas[:], sbas_d[:, :])

        # ---------------- y_aspp = proj5(r5) + acc5 ; aspp = bn_relu ----------------
        aspp_sb = [None, None]
        with (
            tc.tile_pool(name="pas", bufs=1) as pas,
            tc.tile_pool(name="pas_ps", bufs=1, space="PSUM") as pasps,
        ):
            wp5 = pas.tile([P, 2, 256], F32R, name="wp5")
            nc.sync.dma_start(
                wp5[:],
                wproj5_d[:, :].rearrange("(t p) c -> p t c", p=P).bitcast(F32R))
            r5t = [None, None]
            for cti in range(2):
                r5t[cti] = pas.tile([P, HW], F32R, name=f"r5t{cti}")
                nc.sync.dma_start(r5t[cti][:],
                                  r5_d[cti * P:(cti + 1) * P, :].bitcast(F32R))
            for ct in range(2):
                aspp_sb[ct] = glob.tile([P, HW], F32, name=f"aspp{ct}")
            for pb in range(NB):
                for cto in range(2):
                    ps5 = pasps.tile([P, 512], F32, name=f"ps5_{cto}")
                    for cti in range(2):
                        nc.tensor.matmul(ps5[:], wp5[:, cti, cto * P:(cto + 1) * P],
                                         r5t[cti][:, pb * 512:(pb + 1) * 512],
                                         start=(cti == 0), stop=(cti == 1))
                    a5t = pas.tile([P, 512], F32, name="a5t", bufs=4)
                    nc.sync.dma_start(
                        a5t[:], acc5_d[cto * P:(cto + 1) * P, pb * 512:(pb + 1) * 512])
                    ya = pas.tile([P, 512], F32, name="ya", bufs=4)
                    nc.vector.scalar_tensor_tensor(
                        ya[:], ps5[:], 1.0, a5t[:], op0=ALU.mult, op1=ALU.add)
                    nc.scalar.activation(aspp_sb[cto][:, pb * 512:(pb + 1) * 512],
                                         ya[:], AF.Relu,
                                         bias=sbas[:, 2 * cto + 1:2 * cto + 2],
                                         scale=sbas[:, 2 * cto:2 * cto + 1])

        # ---------------- bilinear x2 upsample -> width-padded up_dr ----------------
        with tc.tile_pool(name="pup", bufs=1) as pup:
            up4 = up_dr.rearrange("p (h w) -> p h w", h=128, w=130)
            for ct in range(2):
                aspp = aspp_sb[ct]
                src_m = aspp.rearrange("p (h one w) -> p h one w", one=1, w=W64)
                s25 = pup.tile([P, HW], F32, name="s25")
                nc.vector.tensor_scalar_mul(s25[:], aspp[:], 0.25)
                s25_m = s25.rearrange("p (h one w) -> p h one w", one=1, w=W64)
                hm = pup.tile([P, 128 * W64], F32, name="hm")
                hme = hm.rearrange("p (h two w) -> p h two w", two=2, w=W64)
                nc.scalar.activation(hme[:, 0:1, 0:1, :], src_m[:, 0:1, :, :],
                                     AF.Identity)
                nc.vector.scalar_tensor_tensor(
                    hme[:, 1:W64, 0:1, :], src_m[:, 1:W64, :, :], 0.75,
                    s25_m[:, 0:W64 - 1, :, :], op0=ALU.mult, op1=ALU.add)
                nc.vector.scalar_tensor_tensor(
                    hme[:, 0:W64 - 1, 1:2, :], src_m[:, 0:W64 - 1, :, :], 0.75,
                    s25_m[:, 1:W64, :, :], op0=ALU.mult, op1=ALU.add)
                nc.scalar.activation(hme[:, W64 - 1:W64, 1:2, :],
                                     src_m[:, W64 - 1:W64, :, :], AF.Identity)
                s25w = pup.tile([P, 128 * W64], F32, name="s25w")
                nc.vector.tensor_scalar_mul(s25w[:], hm[:], 0.25)
                hm_m = hm.rearrange("p (h w one) -> p h w one", one=1, h=128, w=W64)
                sw_m = s25w.rearrange("p (h w one) -> p h w one", one=1, h=128, w=W64)
                for rc in range(8):
                    r0, r1 = rc * 16, rc * 16 + 16
                    wo = pup.tile([P, 16, 130], F32, name="wo", bufs=3)
                    nc.vector.memset(wo[:, :, 0:1], 0.0)
                    nc.vector.memset(wo[:, :, 129:130], 0.0)
                    woe = wo[:, :, 1:129].rearrange("p h (w two) -> p h w two", two=2)
                    nc.scalar.activation(woe[:, :, 0:1, 0:1], hm_m[:, r0:r1, 0:1, :],
                                         AF.Identity)
                    nc.vector.scalar_tensor_tensor(
                        woe[:, :, 1:W64, 0:1], hm_m[:, r0:r1, 1:W64, :], 0.75,
                        sw_m[:, r0:r1, 0:W64 - 1, :], op0=ALU.mult, op1=ALU.add)
                    nc.vector.scalar_tensor_tensor(
                        woe[:, :, 0:W64 - 1, 1:2], hm_m[:, r0:r1, 0:W64 - 1, :], 0.75,
                        sw_m[:, r0:r1, 1:W64, :], op0=ALU.mult, op1=ALU.add)
                    nc.scalar.activation(woe[:, :, W64 - 1:W64, 1:2],
                                         hm_m[:, r0:r1, W64 - 1:W64, :], AF.Identity)
                    nc.sync.dma_start(up4[ct * P:(ct + 1) * P, r0:r1, :], wo[:])

        # ---------------- cls1: 3x3 conv, pad 1, over 304 channels ----------------
        c1_stats = glob.tile([P, 4], F32, name="c1_stats")
        c1S = glob.tile([P, 64], F32, name="c1S")
        c1Q = glob.tile([P, 64], F32, name="c1Q")
        lowp4 = lowp_d[:, :].rearrange("p (h w) -> p h w", h=128, w=130)
        up4r = up_dr.rearrange("p (h w) -> p h w", h=128, w=130)
        with (
            tc.tile_pool(name="pc1", bufs=1) as pc1,
            tc.tile_pool(name="pc1_ps", bufs=1, space="PSUM") as pc1ps,
        ):
            wc1 = []
            roff = 0
            for kt, rows in enumerate(KTC):
                wt = pc1.tile([rows, 9, 256], F32R, name=f"wc1_{kt}")
                nc.sync.dma_start(
                    wt[:],
                    wcls1_d[roff:roff + rows, :]
                    .rearrange("p (t c) -> p t c", t=9).bitcast(F32R))
                wc1.append(wt)
                roff += rows
            for rb2 in range(8):
                yy0 = rb2 * 16
                slo = max(0, yy0 - 1)
                shi = min(128, yy0 + 17)
                nrows = shi - slo
                slabs = []
                for kt, rows in enumerate(KTC):
                    sl = pc1.tile([rows, 18, 130], F32R, name=f"slab{kt}", bufs=2)
                    if kt < 2:
                        src = up4r[kt * P:(kt + 1) * P, slo:shi, :]
                    else:
                        src = lowp4[:, slo:shi, :]
                    nc.sync.dma_start(sl[:, 0:nrows, :], src.bitcast(F32R))
                    slabs.append(sl)
                for sub in range(4):
                    r0 = yy0 + sub * 4  # global output row
                    for cto in range(2):
                        psc = pc1ps.tile([P, 512], F32, name=f"psc{cto}")
                        ps3 = psc.rearrange("p (h w) -> p h w", h=4)
                        taps = []
                        for iy, dy in enumerate((-1, 0, 1)):
                            ylo = max(r0, -dy)
                            yhi = min(r0 + 4, 128 - dy)
                            if yhi <= ylo:
                                continue
                            for ix, dx in enumerate((-1, 0, 1)):
                                for kt in range(3):
                                    taps.append((iy * 3 + ix, kt, dy, dx, ylo, yhi))
                        nt = len(taps)
                        for i, (t, kt, dy, dx, ylo, yhi) in enumerate(taps):
                            sl3 = slabs[kt]
                            nc.tensor.matmul(
                                ps3[:, ylo - r0:yhi - r0, :],
                                wc1[kt][:, t, cto * P:(cto + 1) * P],
                                sl3[:, ylo + dy - slo:yhi + dy - slo,
                                    1 + dx:129 + dx],
                                start=(i == 0), stop=(i == nt - 1))
                        ci = cto * 32 + rb2 * 4 + sub
                        stc = pc1.tile([P, 512], F32, name="stc1", bufs=4)
                        nc.scalar.activation(stc[:], psc[:], AF.Identity,
                                             accum_out=c1S[:, ci:ci + 1])
                        sqc = pc1.tile([P, 512], F32, name="sqc1", bufs=4)
                        nc.scalar.activation(sqc[:], psc[:], AF.Square,
                                             accum_out=c1Q[:, ci:ci + 1])
                        nc.sync.dma_start(
                            ycls1_dr[cto * P:(cto + 1) * P, r0 * 128:(r0 + 4) * 128],
                            stc[:])
            for ct in range(2):
                nc.vector.reduce_sum(c1_stats[:, ct:ct + 1],
                                     c1S[:, ct * 32:(ct + 1) * 32], axis=AX.X)
                nc.vector.reduce_sum(c1_stats[:, 2 + ct:3 + ct],
                                     c1Q[:, ct * 32:(ct + 1) * 32], axis=AX.X)
        nc.sync.dma_start(ar_in[:], c1_stats[:])
        nc.gpsimd.collective_compute(
            "AllReduce", ALU.add, replica_groups=RG,
            ins=[ar_in.opt()], outs=[ar_out.opt()])
        c1_statsg = glob.tile([P, 4], F32, name="c1_statsg")
        nc.sync.dma_start(c1_statsg[:], ar_out[:])

        # ---------------- h = bn_relu(ycls1); out = wcls2.T @ h + b ----------------
        with (
            tc.tile_pool(name="pc2", bufs=1) as pc2,
            tc.tile_pool(name="pc2_ps", bufs=1, space="PSUM") as pc2ps,
        ):
            sb_c1 = glob.tile([P, 4], F32, name="sb_c1")
            for ct in range(2):
                m = pc2.tile([P, 1], F32, name=f"m_c1{ct}")
                v = pc2.tile([P, 1], F32, name=f"v_c1{ct}")
                t = pc2.tile([P, 1], F32, name=f"t_c1{ct}")
                nc.vector.tensor_scalar_mul(m[:], c1_statsg[:, ct:ct + 1], 1.0 / N_LOW)
                nc.vector.tensor_scalar_mul(v[:], c1_statsg[:, 2 + ct:3 + ct],
                                            1.0 / N_LOW)
                nc.vector.tensor_mul(t[:], m[:], m[:])
                nc.vector.tensor_sub(v[:], v[:], t[:])
                nc.vector.tensor_scalar_add(v[:], v[:], EPS)
                nc.vector.reciprocal(t[:], v[:])
                nc.scalar.activation(sb_c1[:, 2 * ct:2 * ct + 1], t[:], AF.Sqrt)
                nc.vector.tensor_mul(sb_c1[:, 2 * ct + 1:2 * ct + 2], m[:],
                                     sb_c1[:, 2 * ct:2 * ct + 1])
                nc.vector.tensor_scalar_mul(sb_c1[:, 2 * ct + 1:2 * ct + 2],
                                            sb_c1[:, 2 * ct + 1:2 * ct + 2], -1.0)
            wc2 = pc2.tile([P, 2, 21], F32R, name="wc2")
            nc.sync.dma_start(
                wc2[:], wcls2_d[:, :].rearrange("(t p) c -> p t c", p=P).bitcast(F32R))
            bct = pc2.tile([21, 1], F32, name="bct")
            nc.sync.dma_start(bct[:], bcls2_d[:, :])
            for pb in range(32):
                hts = []
                for kt in range(2):
                    yt = pc2.tile([P, 512], F32, name="yc1_in", bufs=4)
                    nc.sync.dma_start(
                        yt[:], ycls1_dr[kt * P:(kt + 1) * P, pb * 512:(pb + 1) * 512])
                    ht = pc2.tile([P, 512], F32R, name="h_t", bufs=4)
                    nc.scalar.activation(ht[:], yt[:], AF.Relu,
                                         bias=sb_c1[:, 2 * kt + 1:2 * kt + 2],
                                         scale=sb_c1[:, 2 * kt:2 * kt + 1])
                    hts.append(ht)
                ps2 = pc2ps.tile([P, 512], F32, name="ps2")
                for kt in range(2):
                    nc.tensor.matmul(ps2[0:21, :], wc2[:, kt, :], hts[kt][:],
                                     start=(kt == 0), stop=(kt == 1))
                ot = pc2.tile([P, 512], F32, name="out_t", bufs=4)
                nc.scalar.activation(ot[0:21, :], ps2[0:21, :], AF.Identity,
                                     bias=bct[:])
                nc.sync.dma_start(out_o[:, pb * 512:(pb + 1) * 512], ot[0:21, :])

        glob_cm.__exit__(None, None, None)
        dram_cm.__exit__(None, None, None)
    nc.compile()
    return nc


# --------------------------------------------------------------------------
# host-side PSO (bit-replica of the reference, jax on CPU)
# --------------------------------------------------------------------------

def _pso_select_host(ch_sums, init_particles, rands):
    import jax
    import jax.numpy as jnp
    from jax import lax

    INERTIA, COG, SOC = 0.7, 1.5, 1.5
    num_filters = 256

    def pso(channel_sums, init_p, rs):
        particles = init_p.astype(jnp.float32)
        velocities = jnp.zeros_like(particles)

        def obj(p):
            return channel_sums[jnp.floor(p).astype(jnp.int32)].sum()

        best_pos = particles
        best_scores = jax.vmap(obj)(particles)
        gi = jnp.argmin(best_scores)
        g_pos, g_score = particles[gi], best_scores[gi]
        n_particles = particles.shape[0]

        def iter_step(carry, rand_it):
            def particle_step(c, i):
                particles, velocities, best_pos, best_scores, g_pos, g_score = c
                r1, r2 = rand_it[i, 0], rand_it[i, 1]
                v = (INERTIA * velocities[i]
                     + COG * r1 * (best_pos[i] - particles[i])
                     + SOC * r2 * (g_pos - particles[i]))
                p = jnp.clip(particles[i] + v, 0.0, num_filters - 1.0)
                fit = obj(p)
                better = fit < best_scores[i]
                best_pos = best_pos.at[i].set(jnp.where(better, p, best_pos[i]))
                best_scores = best_scores.at[i].set(
                    jnp.where(better, fit, best_scores[i]))
                gbetter = fit < g_score
                g_pos = jnp.where(gbetter, p, g_pos)
                g_score = jnp.where(gbetter, fit, g_score)
                particles = particles.at[i].set(p)
                velocities = velocities.at[i].set(v)
                return (particles, velocities, best_pos, best_scores, g_pos,
                        g_score), None
            c, _ = lax.scan(particle_step, carry, jnp.arange(n_particles))
            return c, None

        carry = (particles, velocities, best_pos, best_scores, g_pos, g_score)
        carry, _ = lax.scan(iter_step, carry, rs)
        return jnp.floor(carry[4]).astype(jnp.int32)

    cpu = jax.devices("cpu")[0]
    with jax.default_device(cpu):
        best = jax.jit(pso)(jnp.asarray(ch_sums), jnp.asarray(init_particles),
                            jnp.asarray(rands))
        return np.asarray(best)


# --------------------------------------------------------------------------
# orchestration
# --------------------------------------------------------------------------

_NC_CACHE = {}


def _get_nc(stage):
    if stage not in _NC_CACHE:
        _NC_CACHE[stage] = build_stage_a() if stage == "a" else build_stage_b()
    return _NC_CACHE[stage]


def _run(nc, in_maps):
    res = run_bass_kernel_spmd(nc, in_maps, core_ids=list(range(NCORES)),
                               trace=_trace)
    return res


def kernel(feature_out, feature_low, w_low, w_a0, w_a1, w_a2, w_a3, w_pool,
           w_max, w_sel, w_proj, w_cls1, w_cls2, b_cls2, rand_scalars,
           init_particles):
    feature_out = np.ascontiguousarray(np.asarray(feature_out, dtype=np.float32))
    feature_low = np.ascontiguousarray(np.asarray(feature_low, dtype=np.float32))

    def t2d(w):  # [cout, cin, 1, 1] -> [cin, cout]
        return np.ascontiguousarray(
            np.asarray(w, np.float32).reshape(w.shape[0], w.shape[1]).T)

    wdil = np.stack([np.asarray(w, np.float32) for w in (w_a1, w_a2, w_a3)])
    # [3, 256, 2048, 3, 3] -> [2048, 3, 9, 256] -> [2048, 3*9*256]
    wdil = np.ascontiguousarray(
        wdil.transpose(2, 0, 3, 4, 1).reshape(2048, 3, 9, 256).reshape(2048, -1))
    wa0 = t2d(w_a0)
    wmax = t2d(w_max)
    wpool = t2d(w_pool)
    wlow = t2d(w_low)
    wproj = t2d(w_proj)                       # [1536, 256]
    wsel = t2d(w_sel)                         # [9, 256]
    # [256, 304, 3, 3] -> [304, 3, 3, 256] -> [304, 9*256]; rows reordered to
    # the device slab order (up 256 first, then low 48)
    _wc1 = np.asarray(w_cls1, np.float32).transpose(1, 2, 3, 0).reshape(304, -1)
    wcls1 = np.ascontiguousarray(np.concatenate([_wc1[48:304], _wc1[0:48]], axis=0))
    wcls2 = t2d(w_cls2)                       # [256, 21]
    bcls2 = np.ascontiguousarray(np.asarray(b_cls2, np.float32).reshape(21, 1))

    nc_a = _get_nc("a")
    in_maps_a = []
    for i in range(NCORES):
        in_maps_a.append({
            "x": feature_out[i].reshape(2048, HW),
            "wdil": wdil, "wa0": wa0, "wmax": wmax, "wpool": wpool,
            "xlow": feature_low[i].reshape(256, 16384),
            "wlow": wlow, "wproj": wproj,
        })
    res_a = _run(nc_a, in_maps_a)
    if _trace:
        LAST_EXEC_NS["a"] = res_a.exec_time_ns

    xm = np.stack([res_a.results[i]["xm_o"] for i in range(NCORES)])   # [8,256,4096]
    chs = np.sum([res_a.results[i]["chs_o"][:, 0] for i in range(NCORES)], axis=0)
    best = _pso_select_host(chs.astype(np.float32),
                            np.asarray(init_particles, np.int32),
                            np.asarray(rand_scalars, np.float32))
    xsel = np.ascontiguousarray(xm[:, best, :])                        # [8,9,4096]

    # host: sel conv + its BN (tiny), r5, and aspp BN stats
    acc5 = np.stack([res_a.results[i]["acc5_o"] for i in range(NCORES)])
    y_sel = np.einsum("ck,bcp->bkp", wsel, xsel, optimize=True)        # [8,256,4096]
    m = y_sel.mean(axis=(0, 2), dtype=np.float64)
    v = (y_sel.astype(np.float64) ** 2).mean(axis=(0, 2)) - m * m
    sca = (1.0 / np.sqrt(v + EPS))
    r5 = np.maximum((y_sel - m[None, :, None].astype(np.float32))
                    * sca[None, :, None].astype(np.float32), 0.0).astype(np.float32)
    wproj5 = np.ascontiguousarray(wproj[1280:1536])
    y_aspp = acc5 + np.einsum("ck,bcp->bkp", wproj5, r5, optimize=True)
    ma = y_aspp.mean(axis=(0, 2), dtype=np.float64)
    va = (y_aspp.astype(np.float64) ** 2).mean(axis=(0, 2)) - ma * ma
    s_as = (1.0 / np.sqrt(va + EPS)).astype(np.float32)                # [256]
    b_as = (-ma * (1.0 / np.sqrt(va + EPS))).astype(np.float32)
    sbas = np.zeros((128, 4), np.float32)
    for ct in range(2):
        sbas[:, 2 * ct] = s_as[ct * 128:(ct + 1) * 128]
        sbas[:, 2 * ct + 1] = b_as[ct * 128:(ct + 1) * 128]

    nc_b = _get_nc("b")
    in_maps_b = []
    for i in range(NCORES):
        lowp = np.zeros((48, 128, 130), np.float32)
        lowp[:, :, 1:129] = res_a.results[i]["low_o"].reshape(48, 128, 128)
        in_maps_b.append({
            "r5": r5[i],
            "acc5": acc5[i],
            "lowp": lowp.reshape(48, 128 * 130),
            "sbas": sbas,
            "wproj5": wproj5,
            "wcls1": wcls1, "wcls2": wcls2, "bcls2": bcls2,
        })
    res_b = _run(nc_b, in_maps_b)
    if _trace:
        LAST_EXEC_NS["b"] = res_b.exec_time_ns

    out = np.stack([res_b.results[i]["out_o"].reshape(21, 128, 128)
                    for i in range(NCORES)])
    return out.astype(np.float32)
